# revision 1
# baseline (speedup 1.0000x reference)
"""Trainium2 Bass kernel for nn_AutoRegressive_45234595562178.

Same width-schedule / sorted-column / snapshot-gather skeleton as the
baseline, with a restructured LSTM cell that cuts the DVE (bottleneck
engine) from 5 ops/lane/step to 2:

  - per-lane gate row orders (lane0 [o,f,i,g], lane1 [f,i,o,g]) make every
    same-space STT base-partition-legal without the g-alignment copy;
  - ONE stacked STT computes u=(f~+1)*C2 and v=(i~+1)*g~ together
    ([64,W], in0 = [f;i] rows of tg (SBUF), in1 = [C2; g~] half of a PSUM
    tile -- mixed SBUF/PSUM operands are exempt from the equal-base rule);
  - the c' combine (0.5u+v) and the g~ staging both move to the idle PE as
    tiny matmuls into a shared PSUM tile cb = [c'0; g0; c'1; g1];
  - ONE shared activation computes tanh(0.5*cb) for both lanes (junk g
    rows harmless), aligned so each lane's h' STT is base-legal;
  - snapshots run on the idle GPSIMD engine (h windows and [u;v] windows;
    C2 of captured columns is reconstructed post-gather as 0.5u+v);
  - decode predictions use one shared [13, *] activation + one DMA per
    step (junk gap columns are masked by the host anyway).

State: h' = 2h in per-lane SBUF tiles [32, HL]; C2 = 2c lives only in the
rotating PSUM cb tiles.
"""

import numpy as np
import ml_dtypes

D_IN, D_H, B, T, MAX_OUT, NCORES = 13, 32, 4096, 512, 256, 8
BPC = B // NCORES
H = BPC // 2
G4 = 4 * D_H
SNAP_PAD = 4
X_CHUNK = 64

BF16 = ml_dtypes.bfloat16

# gate row orders per lane: partition base of each gate block in tg/pg
ROWS = [
    dict(f=0, i=32, o=64, g=96),   # [f, i, o, g] -- [f;i] 64-aligned for
    dict(f=0, i=32, o=64, g=96),   # the stacked UV read; tct lands at o's
]                                  # base so the h' STT is base-legal


def make_schedules(lengths, out_steps, L=2):
    """Identical to the baseline kernel."""
    HL = BPC // L

    def r4(n):
        return min(HL, -(-n // 4) * 4)

    steps = np.clip(np.asarray(lengths).astype(np.int64), 1, T)
    dec = np.clip(np.asarray(out_steps).astype(np.int64), 1, MAX_OUT)

    order = np.argsort(-steps, kind="stable")
    assign = np.stack(
        [np.concatenate([order[c::NCORES][l::L] for l in range(L)])
         for c in range(NCORES)])
    steps_pc = steps[assign]

    Tmax = int(steps.max())
    counts = np.bincount(steps, minlength=T + 2)
    surv = B - np.cumsum(counts)
    Wh = np.array([r4(-(-int(surv[t]) // (NCORES * L))) for t in range(Tmax)],
                  np.int64)
    tgrid = np.arange(Tmax)[:, None]
    for c in range(NCORES):
        for l in range(L):
            scol = steps_pc[c, l * HL:(l + 1) * HL]
            n_ct = (scol[None, :] > tgrid).sum(1)
            assert np.all(Wh >= n_ct), "width schedule violates actives"

    Whnext = np.append(Wh[1:], 0)
    lo = np.maximum(0, Whnext - SNAP_PAD)
    wwin = Wh - lo
    off = np.concatenate([[0], np.cumsum(L * wwin)])
    S = int(off[-1])
    S_pad = -(-S // 128) * 128

    slot = np.zeros((NCORES, BPC), np.int64)
    for c in range(NCORES):
        for l in range(L):
            scol = steps_pc[c, l * HL:(l + 1) * HL]
            tprime = scol - 1
            j = np.arange(HL)
            assert np.all(j >= lo[tprime]) and np.all(j < Wh[tprime])
            slot[c, l * HL:(l + 1) * HL] = (
                off[tprime] + l * wwin[tprime] + (j - lo[tprime]))

    dec_pc = dec[assign]
    dorder = np.zeros((NCORES, BPC), np.int64)
    for c in range(NCORES):
        didx = np.argsort(-dec_pc[c], kind="stable")
        dorder[c] = np.concatenate([didx[l::L] for l in range(L)])
    dec_at = np.take_along_axis(dec_pc, dorder, axis=1)
    Ms = np.zeros((L, MAX_OUT), np.int64)
    Ms[:, 0] = HL
    for s in range(1, MAX_OUT):
        for l in range(L):
            Ms[l, s] = r4(int((dec_at[:, l * HL:(l + 1) * HL] > s).sum(1).max()))
    for l in range(1, L):
        assert np.all(Ms[l - 1] >= Ms[l])
    Smax = int(np.nonzero(Ms[0])[0].max()) + 1

    pmat = np.zeros((NCORES, S_pad, BPC), np.float32)
    for c in range(NCORES):
        pmat[c, slot[c][dorder[c]], np.arange(BPC)] = 1.0

    return dict(
        steps=steps, dec=dec, assign=assign, steps_pc=steps_pc, Tmax=Tmax,
        Wh=Wh, lo=lo, wwin=wwin, off=off, S=S, S_pad=S_pad, slot=slot,
        dorder=dorder, dec_pc=dec_pc, Ms=Ms, Smax=Smax, pmat=pmat, L=L, HL=HL,
    )


def prep_weights(W_ih, W_hh, b_ih, b_hh, Wd, bd):
    """Scale-folded weights with PER-LANE gate row (weight column) orders."""
    rs = np.ones(G4, np.float32) * 0.5
    rs[64:96] = 1.0  # g rows keep full scale in natural [i,f,g,o] order
    Wx_n = (rs[:, None] * np.asarray(W_ih, np.float32)).T        # [13, 128]
    Wh_n = (rs[:, None] * 0.5 * np.asarray(W_hh, np.float32)).T  # [32, 128]
    bias_n = rs * (np.asarray(b_ih, np.float32) + np.asarray(b_hh, np.float32))

    nat = dict(i=0, f=32, g=64, o=96)
    perms = []
    for l in range(2):
        p = np.zeros(G4, np.int64)
        for gname in ("i", "f", "g", "o"):
            p[ROWS[l][gname]:ROWS[l][gname] + 32] = np.arange(
                nat[gname], nat[gname] + 32)
        perms.append(p)

    Wx = [np.ascontiguousarray(Wx_n[:, p]) for p in perms]
    Wh_ = [np.ascontiguousarray(Wh_n[:, p]) for p in perms]
    bias = [np.ascontiguousarray(bias_n[p][:, None]) for p in perms]
    Wdp = (0.5 * np.asarray(Wd, np.float32)).T  # [32, 13]
    bdp = np.asarray(bd, np.float32)[:, None]   # [13, 1]

    compA = np.zeros((64, 32), np.float32)      # [u; v] -> 0.5 u + v
    compA[0:32] = 0.5 * np.eye(32)
    compA[32:64] = np.eye(32)
    selG = []
    for l in range(2):
        s = np.zeros((G4, 32), np.float32)
        s[ROWS[l]["g"]:ROWS[l]["g"] + 32] = np.eye(32)
        selG.append(np.ascontiguousarray(s))
    return Wx, Wh_, bias, Wdp, bdp, compA, selG


def _split_sync_waits(m):
    import bass_rust
    import concourse.mybir as mybir
    ctr = [0]
    for fn in m.functions:
        for bb in fn.blocks:
            out_list = []
            changed = False
            for inst in bb.instructions:
                si = inst.sync_info
                waits = list(si.on_wait) if si is not None else []
                if len(waits) > 1:
                    changed = True
                    for w in waits[:-1]:
                        ctr[0] += 1
                        nop = mybir.InstNoOp(
                            name=f"wsplit-{ctr[0]}", ins=[], outs=[])
                        nop.engine = inst.engine
                        nop.sync_info = bass_rust.SyncInfo(
                            on_wait=[w], on_update=[])
                        out_list.append(nop)
                    si.on_wait = waits[-1:]
                out_list.append(inst)
            if changed:
                bb.instructions = out_list


def _build_program(sch, reps=1):
    import concourse.bass as bass
    import concourse.mybir as mybir
    from concourse.tile import TileContext

    fp32 = mybir.dt.float32
    bf16 = mybir.dt.bfloat16
    ADD = mybir.AluOpType.add
    MULT = mybir.AluOpType.mult
    TANH = mybir.ActivationFunctionType.Tanh
    IDENT = mybir.ActivationFunctionType.Identity

    Tmax, Wh, lo, wwin, off = (sch["Tmax"], sch["Wh"], sch["lo"], sch["wwin"],
                               sch["off"])
    S_pad, Ms, Smax = sch["S_pad"], sch["Ms"], sch["Smax"]
    L, HL = sch["L"], sch["HL"]
    KCH = S_pad // 128

    nc = bass.Bass("TRN2", target_bir_lowering=False)
    xt = nc.dram_tensor("xt", [T, D_IN, BPC], bf16, kind="ExternalInput")
    wx_d = [nc.dram_tensor(f"wx{l}", [D_IN, G4], bf16, kind="ExternalInput")
            for l in range(L)]
    wh_d = [nc.dram_tensor(f"wh{l}", [D_H, G4], bf16, kind="ExternalInput")
            for l in range(L)]
    bg_d = [nc.dram_tensor(f"bias{l}", [G4, 1], fp32, kind="ExternalInput")
            for l in range(L)]
    ca_d = nc.dram_tensor("compA", [64, 32], bf16, kind="ExternalInput")
    sg_d = [nc.dram_tensor(f"selG{l}", [G4, 32], bf16, kind="ExternalInput")
            for l in range(L)]
    wd_d = nc.dram_tensor("wd", [D_H, D_IN], bf16, kind="ExternalInput")
    bd_d = nc.dram_tensor("bd", [D_IN, 1], fp32, kind="ExternalInput")
    pm_d = nc.dram_tensor("pmat", [S_pad, BPC], bf16, kind="ExternalInput")
    id_d = nc.dram_tensor("ident", [64, 64], bf16, kind="ExternalInput")
    out_d = nc.dram_tensor("out", [MAX_OUT, D_IN, BPC], fp32,
                           kind="ExternalOutput")

    with TileContext(nc) as tc:
        with (
            tc.tile_pool(name="consts", bufs=1) as cpool,
            tc.tile_pool(name="state", bufs=1) as spool,
            tc.tile_pool(name="xin", bufs=2) as xpool,
            tc.tile_pool(name="gates", bufs=5) as gpool,
            tc.tile_pool(name="vtmp", bufs=5) as vpool,
            tc.tile_pool(name="outs", bufs=4) as opool,
            tc.tile_pool(name="pmchunk", bufs=2) as pmpool,
            tc.tile_pool(name="snapT", bufs=3) as stpool,
            tc.tile_pool(name="pgates", bufs=3, space="PSUM") as pgpool,
            tc.tile_pool(name="pcb", bufs=1, space="PSUM") as cbpool,
            tc.tile_pool(name="pacc", bufs=1, space="PSUM") as papool,
            tc.tile_pool(name="ptr", bufs=1, space="PSUM") as ptpool,
        ):
            def emit_body():
                wxF, whF, biasG, selGt = [], [], [], []
                for l in range(L):
                    t_ = cpool.tile([D_IN, G4], bf16, name=f"wx{l}")
                    nc.sync.dma_start(t_[:], wx_d[l][:])
                    wxF.append(t_)
                    t_ = cpool.tile([D_H, G4], bf16, name=f"wh{l}")
                    nc.sync.dma_start(t_[:], wh_d[l][:])
                    whF.append(t_)
                    t_ = cpool.tile([G4, 1], fp32, name=f"bias{l}")
                    nc.sync.dma_start(t_[:], bg_d[l][:])
                    biasG.append(t_)
                    t_ = cpool.tile([G4, 32], bf16, name=f"selG{l}")
                    nc.sync.dma_start(t_[:], sg_d[l][:])
                    selGt.append(t_)
                compAt = cpool.tile([64, 32], bf16)
                nc.sync.dma_start(compAt[:], ca_d[:])
                wd_sb = cpool.tile([D_H, D_IN], bf16)
                nc.sync.dma_start(wd_sb[:], wd_d[:])
                bd_sb = cpool.tile([D_IN, 1], fp32)
                nc.sync.dma_start(bd_sb[:], bd_d[:])
                id_sb = cpool.tile([64, 64], bf16)
                nc.sync.dma_start(id_sb[:], id_d[:])

                hts = []
                for l in range(L):
                    ht = spool.tile([D_H, HL], bf16, name=f"ht{l}")
                    nc.vector.memset(ht[:], 0.0)
                    hts.append(ht)
                cbs = []
                for l in range(L):
                    cb_l = cbpool.tile([64, HL], fp32, name=f"cb{l}")
                    nc.vector.memset(cb_l[:], 0.0)
                    cbs.append(cb_l)
                snapH = spool.tile([64, S_pad], bf16)
                nc.vector.memset(snapH[:], 0.0)
                snapUV = spool.tile([64, S_pad], bf16)
                nc.vector.memset(snapUV[:], 0.0)

                def step(jobs):
                    """jobs: (W, lhsT_x, rhs_x, lane). One LSTM step. Each
                    lane's single-buffered cb PSUM tile holds [C2; g~]; g
                    rows are staged by PE from the CURRENT step's tg, C2
                    rows were produced by the PREVIOUS step's compA matmul
                    (read-then-overwrite keeps single buffering correct)."""
                    jobs = [j for j in jobs if j[0]]
                    pg = pgpool.tile([G4, BPC], fp32, tag="pg")
                    # NOTE: keep each lane's accumulation group closed before
                    # opening the next -- interleaved open groups on one PSUM
                    # bank corrupt each other (the later start resets it).
                    for W, kx, rx, l in jobs:
                        nc.tensor.matmul(pg[:, l * HL:l * HL + W], kx, rx,
                                         start=True, stop=False)
                        nc.tensor.matmul(pg[:, l * HL:l * HL + W], whF[l][:],
                                         hts[l][:, :W], start=False, stop=True)
                    tgs = {}
                    for W, kx, rx, l in jobs:
                        tg = gpool.tile([G4, HL], bf16, tag=f"tg{l}")
                        nc.scalar.activation(tg[:, :W],
                                             pg[:, l * HL:l * HL + W], TANH,
                                             bias=biasG[l][:])
                        tgs[l] = tg
                    for W, kx, rx, l in jobs:
                        nc.tensor.matmul(cbs[l][32:64, :W],
                                         selGt[l][:], tgs[l][:, :W],
                                         start=True, stop=True)
                    uvs = {}
                    for W, kx, rx, l in jobs:
                        fi = ROWS[l]["f"]
                        uv = vpool.tile([64, HL], bf16, tag=f"uv{l}")
                        nc.vector.scalar_tensor_tensor(
                            uv[:, :W], tgs[l][fi:fi + 64, :W], 1.0,
                            cbs[l][:, :W], ADD, MULT)
                        uvs[l] = uv
                    for W, kx, rx, l in jobs:
                        nc.tensor.matmul(cbs[l][0:32, :W],
                                         compAt[:], uvs[l][:, :W],
                                         start=True, stop=True)
                    tcts = {}
                    for W, kx, rx, l in jobs:
                        to = ROWS[l]["o"]
                        tct = vpool.tile([G4, HL], bf16, tag=f"tct{l}")
                        nc.scalar.activation(tct[to:to + 32, :W],
                                             cbs[l][0:32, :W],
                                             TANH, scale=0.5)
                        tcts[l] = tct
                    for W, kx, rx, l in jobs:
                        o = ROWS[l]["o"]
                        nc.vector.scalar_tensor_tensor(
                            hts[l][:, :W], tgs[l][o:o + 32, :W], 1.0,
                            tcts[l][o:o + 32, :W], ADD, MULT)
                    return uvs

                # ---- warmup ----
                xc = None
                for t in range(Tmax):
                    if t % X_CHUNK == 0:
                        ch = min(X_CHUNK, Tmax - t)
                        xc = xpool.tile([D_IN, X_CHUNK, BPC], bf16, tag="xc")
                        nc.sync.dma_start(
                            xc[:, :ch, :],
                            xt[t:t + ch].rearrange("t d b -> d t b"))
                    tl = t % X_CHUNK
                    W = int(Wh[t])
                    uvs = step([(W, wxF[l][:], xc[:, tl, l * HL:l * HL + W], l)
                                for l in range(L)])
                    lw, w, o = int(lo[t]), int(wwin[t]), int(off[t])
                    for l in range(L):
                        nc.gpsimd.tensor_copy(
                            snapH[0:32, o + l * w:o + (l + 1) * w],
                            hts[l][:, lw:lw + w])
                        nc.gpsimd.tensor_copy(
                            snapUV[:, o + l * w:o + (l + 1) * w],
                            uvs[l][:, lw:lw + w])

                # ---- gather snapshots into decode order ----
                accH = papool.tile([64, BPC], fp32, name="accH")
                accUV = papool.tile([64, BPC], fp32, name="accUV")
                for k in range(KCH):
                    pm_k = pmpool.tile([128, BPC], bf16, tag="pm")
                    nc.sync.dma_start(pm_k[:], pm_d[128 * k:128 * (k + 1), :])
                    ptH = ptpool.tile([128, 64], bf16, tag="pt")
                    nc.tensor.transpose(ptH[:],
                                        snapH[:, 128 * k:128 * (k + 1)],
                                        id_sb[:])
                    sTH = stpool.tile([128, 64], bf16, tag="sTH")
                    nc.scalar.copy(sTH[:], ptH[:])
                    nc.tensor.matmul(accH[:], sTH[:], pm_k[:],
                                     start=(k == 0), stop=(k == KCH - 1))
                    ptU = ptpool.tile([128, 64], bf16, tag="pt")
                    nc.tensor.transpose(ptU[:],
                                        snapUV[:, 128 * k:128 * (k + 1)],
                                        id_sb[:])
                    sTU = stpool.tile([128, 64], bf16, tag="sTU")
                    nc.scalar.copy(sTU[:], ptU[:])
                    nc.tensor.matmul(accUV[:], sTU[:], pm_k[:],
                                     start=(k == 0), stop=(k == KCH - 1))

                # decode-order states: h into lane tiles; C2 via compA matmul
                # from the gathered [u; v] (copied to SBUF first).
                cpUV = stpool.tile([64, BPC], bf16, name="cpUV")
                nc.scalar.copy(cpUV[:], accUV[:])
                for l in range(L):
                    nc.scalar.copy(hts[l][:], accH[0:32, l * HL:(l + 1) * HL])
                for l in range(L):
                    nc.tensor.matmul(cbs[l][0:32, :], compAt[:],
                                     cpUV[:, l * HL:(l + 1) * HL],
                                     start=True, stop=True)

                # ---- element = h_sel @ Wd.T + bd ----
                pe = pgpool.tile([D_IN, BPC], fp32, tag="pg")
                for l in range(L):
                    nc.tensor.matmul(pe[:, l * HL:(l + 1) * HL], wd_sb[:],
                                     hts[l][:], start=True, stop=True)
                elem32 = spool.tile([D_IN, BPC], fp32)
                nc.scalar.activation(elem32[:], pe[:], IDENT, bias=bd_sb[:])
                elembf = spool.tile([D_IN, BPC], bf16)
                nc.vector.tensor_copy(elembf[:], elem32[:])
                nc.sync.dma_start(out_d[0], elem32[:])

                # ---- autoregressive decode ----
                for s in range(1, Smax):
                    Wl = [int(Ms[l, s]) for l in range(L)]
                    step([(Wl[l], wxF[l][:],
                           elembf[:, l * HL:l * HL + Wl[l]], l)
                          for l in range(L)])
                    pp = pgpool.tile([D_IN, BPC], fp32, tag="pg")
                    po = opool.tile([D_IN, BPC], fp32, tag="po")
                    for l in range(L):
                        if Wl[l]:
                            cs = l * HL
                            nc.tensor.matmul(pp[:, cs:cs + Wl[l]], wd_sb[:],
                                             hts[l][:, :Wl[l]],
                                             start=True, stop=True)
                    span = HL + Wl[1] if Wl[1] else Wl[0]
                    nc.scalar.activation(po[:, :span], pp[:, :span], IDENT,
                                         bias=bd_sb[:])
                    nc.sync.dma_start(out_d[s, :, 0:span], po[:, :span])

            if reps == 1:
                emit_body()
            else:
                with tc.For_i(0, reps, 1):
                    emit_body()

    _split_sync_waits(nc.m)
    return nc


def _host_prep(x, lengths, out_steps, W_ih, W_hh, b_ih, b_hh, Wd, bd):
    x = np.asarray(x, np.float32)
    sch = make_schedules(lengths, out_steps, L=2)
    Wx, Wh_, bias, Wdp, bdp, compA, selG = prep_weights(
        W_ih, W_hh, b_ih, b_hh, Wd, bd)
    ident = np.eye(64, dtype=np.float32).astype(BF16)
    in_maps = []
    for c in range(NCORES):
        xc = np.ascontiguousarray(
            x[sch["assign"][c]].transpose(1, 2, 0)).astype(BF16)
        im = {
            "xt": xc,
            "wd": Wdp.astype(BF16), "bd": bdp,
            "compA": compA.astype(BF16),
            "pmat": np.ascontiguousarray(sch["pmat"][c]).astype(BF16),
            "ident": ident,
        }
        for l in range(2):
            im[f"wx{l}"] = Wx[l].astype(BF16)
            im[f"wh{l}"] = Wh_[l].astype(BF16)
            im[f"bias{l}"] = bias[l]
            im[f"selG{l}"] = selG[l].astype(BF16)
        in_maps.append(im)
    return sch, in_maps


def _assemble(sch, results):
    out = np.zeros((B, MAX_OUT, D_IN), np.float32)
    ar = np.arange(MAX_OUT)
    for c in range(NCORES):
        dev = results[c]["out"]  # [MAX_OUT, D_IN, BPC]
        ids = sch["assign"][c][sch["dorder"][c]]
        valid = ar[:, None] < sch["dec"][ids][None, :]
        dd = np.where(valid[:, None, :], dev, 0.0)
        out[ids] = dd.transpose(2, 0, 1)
    return out


def kernel(x, lengths, out_steps, max_out, W_ih, W_hh, b_ih, b_hh, Wd, bd):
    from concourse.bass_utils import run_bass_kernel_spmd

    assert int(max_out) == MAX_OUT
    sch, in_maps = _host_prep(x, lengths, out_steps, W_ih, W_hh, b_ih, b_hh,
                              Wd, bd)
    nc = _build_program(sch)
    res = run_bass_kernel_spmd(nc, in_maps, core_ids=list(range(NCORES)))
    return _assemble(sch, res.results)


def measure_hw_time(inputs, R=256, tries=5):
    import time
    from concourse.bass_utils import run_bass_kernel_spmd

    sch, in_maps = _host_prep(
        inputs["x"], inputs["lengths"], inputs["out_steps"], inputs["W_ih"],
        inputs["W_hh"], inputs["b_ih"], inputs["b_hh"], inputs["Wd"],
        inputs["bd"])
    cores = list(range(NCORES))
    ncs = {r: _build_program(sch, reps=r) for r in (1, R)}
    for r in (1, R):
        run_bass_kernel_spmd(ncs[r], in_maps, core_ids=cores)
    walls = {1: [], R: []}
    deltas = []
    for _ in range(tries):
        t0 = time.perf_counter()
        run_bass_kernel_spmd(ncs[1], in_maps, core_ids=cores)
        a = time.perf_counter() - t0
        t0 = time.perf_counter()
        run_bass_kernel_spmd(ncs[R], in_maps, core_ids=cores)
        b = time.perf_counter() - t0
        walls[1].append(a)
        walls[R].append(b)
        deltas.append((b - a) / (R - 1))
    pos = [d for d in deltas if d > 0]
    d = min(pos) if pos else (min(walls[R]) - min(walls[1])) / (R - 1)
    return d * 1e9, walls

